# revision 2
# baseline (speedup 1.0000x reference)
import sys

sys.path.insert(0, "/opt/trn_rl_repo")

import numpy as np
import ml_dtypes

import concourse.bass as bass
import concourse.tile as tile
import concourse.mybir as mybir
from concourse import bacc
from concourse.bass_utils import run_bass_kernel_spmd

B, H, NI, NQ = 4096, 1024, 4096, 2048
NCORES = 8
BS = B // NCORES      # 512 batch rows per core
K = H + NI            # 5120 contraction dim
KT = K // 128         # 40 k-tiles
BT = BS // 128        # 4 batch subtiles per core
HH = H // 512         # 2 h-halves

F32 = mybir.dt.float32
BF16 = mybir.dt.bfloat16
AF = mybir.ActivationFunctionType
OP = mybir.AluOpType

_built = None
LAST_RESULT = None


def _build():
    global _built
    if _built is not None:
        return _built

    nc = bacc.Bacc()
    aht_d = nc.dram_tensor("aht", [K, BS], BF16, kind="ExternalInput")
    alt_d = nc.dram_tensor("alt", [K, BS], BF16, kind="ExternalInput")
    wht_d = nc.dram_tensor("wht", [K, H], BF16, kind="ExternalInput")
    wlt_d = nc.dram_tensor("wlt", [K, H], BF16, kind="ExternalInput")
    biasb_d = nc.dram_tensor("biasb", [128, H], F32, kind="ExternalInput")
    wysel_d = nc.dram_tensor("wysel", [BS, H], F32, kind="ExternalInput")
    byt_d = nc.dram_tensor("byt", [128, BT], F32, kind="ExternalInput")
    trut_d = nc.dram_tensor("trut", [128, BT], F32, kind="ExternalInput")
    hid_d = nc.dram_tensor("hidden_out", [BS, H], F32, kind="ExternalOutput")
    predq_d = nc.dram_tensor("predq", [128, BT], F32, kind="ExternalOutput")
    bceq_d = nc.dram_tensor("bceq", [128, BT], F32, kind="ExternalOutput")

    from contextlib import ExitStack

    with tile.TileContext(nc) as tc, ExitStack() as ctx:
        wpool = ctx.enter_context(tc.tile_pool(name="wpool", bufs=2))
        apool = ctx.enter_context(tc.tile_pool(name="apool", bufs=2))
        cpool = ctx.enter_context(tc.tile_pool(name="cpool", bufs=1))
        epool = ctx.enter_context(tc.tile_pool(name="epool", bufs=1))
        pspool = ctx.enter_context(tc.tile_pool(name="pspool", bufs=1, space="PSUM"))

        bias_sb = cpool.tile([128, H], F32, name="bias_sb")
        nc.sync.dma_start(out=bias_sb, in_=biasb_d[:, :])
        wy_sb = []
        for i in range(BT):
            t = cpool.tile([128, H], F32, tag=f"wy{i}", name=f"wy{i}")
            nc.sync.dma_start(out=t, in_=wysel_d[i * 128 : (i + 1) * 128, :])
            wy_sb.append(t)
        byt_sb = cpool.tile([128, BT], F32, name="byt_sb")
        nc.sync.dma_start(out=byt_sb, in_=byt_d[:, :])
        trut_sb = cpool.tile([128, BT], F32, name="trut_sb")
        nc.sync.dma_start(out=trut_sb, in_=trut_d[:, :])

        psums = []
        for i in range(BT * HH):
            psums.append(pspool.tile([128, 512], F32, tag=f"ps{i}", name=f"ps{i}"))

        for kc in range(KT):
            ks = slice(kc * 128, (kc + 1) * 128)
            wh = wpool.tile([128, H], BF16, tag="wh", name="wh")
            nc.sync.dma_start(out=wh, in_=wht_d[ks, :])
            wl = wpool.tile([128, H], BF16, tag="wl", name="wl")
            nc.sync.dma_start(out=wl, in_=wlt_d[ks, :])
            ah = apool.tile([128, BS], BF16, tag="ah", name="ah")
            nc.sync.dma_start(out=ah, in_=aht_d[ks, :])
            al = apool.tile([128, BS], BF16, tag="al", name="al")
            nc.sync.dma_start(out=al, in_=alt_d[ks, :])
            start = kc == 0
            stop = kc == KT - 1
            for bt_i in range(BT):
                ah_s = ah[:, bt_i * 128 : (bt_i + 1) * 128]
                al_s = al[:, bt_i * 128 : (bt_i + 1) * 128]
                # 4 matmuls with ah stationary, then 2 with al stationary
                for hh_i in range(HH):
                    whs = wh[:, hh_i * 512 : (hh_i + 1) * 512]
                    nc.tensor.matmul(
                        psums[bt_i * HH + hh_i], ah_s, whs, start=start, stop=False
                    )
                for hh_i in range(HH):
                    wls = wl[:, hh_i * 512 : (hh_i + 1) * 512]
                    nc.tensor.matmul(
                        psums[bt_i * HH + hh_i], ah_s, wls, start=False, stop=False
                    )
                for hh_i in range(HH):
                    whs = wh[:, hh_i * 512 : (hh_i + 1) * 512]
                    nc.tensor.matmul(
                        psums[bt_i * HH + hh_i], al_s, whs, start=False, stop=stop
                    )

        # epilogue: bias add + tanh + store hidden
        hid_sb = []
        for i in range(BT * HH):
            bt_i, hh_i = divmod(i, HH)
            zb = epool.tile([128, 512], F32, tag=f"zb{i}", name=f"zb{i}")
            nc.vector.tensor_tensor(
                zb, psums[i], bias_sb[:, hh_i * 512 : (hh_i + 1) * 512], OP.add
            )
            hsb = epool.tile([128, 512], F32, tag=f"hid{i}", name=f"hid{i}")
            nc.scalar.activation(hsb, zb, AF.Tanh)
            nc.sync.dma_start(
                out=hid_d[
                    bt_i * 128 : (bt_i + 1) * 128, hh_i * 512 : (hh_i + 1) * 512
                ],
                in_=hsb,
            )
            hid_sb.append(hsb)

        # head: per-row dot(hidden, wy_sel) + b_y_sel -> sigmoid -> bce
        predq_sb = epool.tile([128, BT], F32, name="predq_sb")
        bceq_sb = epool.tile([128, BT], F32, name="bceq_sb")
        scr = epool.tile([128, H], F32, name="scr")
        zall = epool.tile([128, BT], F32, name="zall")
        lp4 = epool.tile([128, BT], F32, name="lp4")
        l14 = epool.tile([128, BT], F32, name="l14")
        d4 = epool.tile([128, BT], F32, name="d4")
        for bt_i in range(BT):
            h0 = hid_sb[bt_i * HH + 0]
            h1 = hid_sb[bt_i * HH + 1]
            nc.vector.tensor_tensor(
                scr[:, 0:512], h0, wy_sb[bt_i][:, 0:512], OP.mult
            )
            nc.vector.tensor_tensor(
                scr[:, 512:1024], h1, wy_sb[bt_i][:, 512:1024], OP.mult
            )
            nc.vector.tensor_reduce(
                zall[:, bt_i : bt_i + 1], scr, mybir.AxisListType.X, OP.add
            )
        nc.vector.tensor_tensor(zall, zall, byt_sb, OP.add)
        nc.scalar.activation(predq_sb, zall, AF.Sigmoid)
        nc.scalar.activation(lp4, predq_sb, AF.Ln)
        nc.scalar.activation(l14, predq_sb, AF.Ln, bias=1.0, scale=-1.0)
        nc.vector.tensor_scalar_max(lp4, lp4, -100.0)
        nc.vector.tensor_scalar_max(l14, l14, -100.0)
        nc.vector.tensor_tensor(d4, lp4, l14, OP.subtract)
        nc.vector.tensor_tensor(d4, d4, trut_sb, OP.mult)
        nc.vector.tensor_tensor(bceq_sb, d4, l14, OP.add)
        nc.sync.dma_start(out=predq_d[:, :], in_=predq_sb)
        nc.sync.dma_start(out=bceq_d[:, :], in_=bceq_sb)

    nc.finalize()
    _built = nc
    return nc


def kernel(state, inputX, inputY, truth, W_t, b_t, W_x, b_x, W_y, b_y):
    global LAST_RESULT
    bf16 = ml_dtypes.bfloat16
    f32 = np.float32

    A = np.concatenate([np.asarray(state, f32), np.asarray(inputX, f32)], axis=1)
    Ah = A.astype(bf16)
    Al = (A - Ah.astype(f32)).astype(bf16)

    Wcat = np.concatenate([np.asarray(W_t, f32), np.asarray(W_x, f32)], axis=1)
    Wh = Wcat.astype(bf16)
    Wl = (Wcat - Wh.astype(f32)).astype(bf16)
    WhT = np.ascontiguousarray(Wh.T)
    WlT = np.ascontiguousarray(Wl.T)

    biasb = np.ascontiguousarray(
        np.broadcast_to((np.asarray(b_t, f32) + np.asarray(b_x, f32)), (128, H))
    )

    q = np.argmax(np.asarray(inputY), axis=1)
    Wy_sel = np.ascontiguousarray(np.asarray(W_y, f32)[q])  # [B, H]
    by_sel = np.asarray(b_y, f32)[q]                        # [B]
    truth_f = np.asarray(truth, f32)

    in_maps = []
    for c in range(NCORES):
        sl = slice(c * BS, (c + 1) * BS)
        in_maps.append(
            {
                "aht": np.ascontiguousarray(Ah[sl].T),
                "alt": np.ascontiguousarray(Al[sl].T),
                "wht": WhT,
                "wlt": WlT,
                "biasb": biasb,
                "wysel": Wy_sel[sl],
                "byt": np.ascontiguousarray(by_sel[sl].reshape(BT, 128).T),
                "trut": np.ascontiguousarray(truth_f[sl].reshape(BT, 128).T),
            }
        )

    nc = _build()
    res = run_bass_kernel_spmd(nc, in_maps, core_ids=list(range(NCORES)))
    LAST_RESULT = res

    hidden = np.empty((B, H), f32)
    pred = np.empty(B, f32)
    bce_total = 0.0
    for c, out in enumerate(res.results):
        sl = slice(c * BS, (c + 1) * BS)
        hidden[sl] = out["hidden_out"]
        pred[sl] = np.asarray(out["predq"]).T.reshape(BS)
        bce_total += np.asarray(out["bceq"], np.float64).sum()
    err = f32(-bce_total)
    return pred, err, hidden


# revision 3
# speedup vs baseline: 1.0323x; 1.0323x over previous
import sys

sys.path.insert(0, "/opt/trn_rl_repo")

import numpy as np
import ml_dtypes

import concourse.bass as bass
import concourse.tile as tile
import concourse.mybir as mybir
from concourse import bacc
from concourse.bass_utils import run_bass_kernel_spmd

B, H, NI, NQ = 4096, 1024, 4096, 2048
NCORES = 8
BS = B // NCORES      # 512 batch rows per core
K = H + NI            # 5120 contraction dim
KT = K // 128         # 40 k-tiles
BT = BS // 128        # 4 batch subtiles per core
HH = H // 512         # 2 h-halves

F32 = mybir.dt.float32
BF16 = mybir.dt.bfloat16
AF = mybir.ActivationFunctionType
OP = mybir.AluOpType

_built = None
LAST_RESULT = None


def _build():
    global _built
    if _built is not None:
        return _built

    nc = bacc.Bacc()
    aht_d = nc.dram_tensor("aht", [K, BS], BF16, kind="ExternalInput")
    alt_d = nc.dram_tensor("alt", [K, BS], BF16, kind="ExternalInput")
    wht_d = nc.dram_tensor("wht", [K, H], BF16, kind="ExternalInput")
    wlt_d = nc.dram_tensor("wlt", [K, H], BF16, kind="ExternalInput")
    biasb_d = nc.dram_tensor("biasb", [128, H], F32, kind="ExternalInput")
    wysel_d = nc.dram_tensor("wysel", [BS, H], F32, kind="ExternalInput")
    hid_d = nc.dram_tensor("hidden_out", [BS, H], F32, kind="ExternalOutput")
    zsum_d = nc.dram_tensor("zsum", [128, BT], F32, kind="ExternalOutput")

    from contextlib import ExitStack

    with tile.TileContext(nc) as tc, ExitStack() as ctx:
        wpool = ctx.enter_context(tc.tile_pool(name="wpool", bufs=2))
        apool = ctx.enter_context(tc.tile_pool(name="apool", bufs=2))
        cpool = ctx.enter_context(tc.tile_pool(name="cpool", bufs=1))
        epool = ctx.enter_context(tc.tile_pool(name="epool", bufs=1))
        pspool = ctx.enter_context(tc.tile_pool(name="pspool", bufs=1, space="PSUM"))

        # persistent loads on the ACT hardware DMA queue so they do not
        # serialize with the k-tile stream on the SP queue
        bias_sb = cpool.tile([128, H], F32, name="bias_sb")
        nc.scalar.dma_start(out=bias_sb, in_=biasb_d[:, :])
        wy_sb = []
        for i in range(BT):
            t = cpool.tile([128, H], F32, tag=f"wy{i}", name=f"wy{i}")
            nc.scalar.dma_start(out=t, in_=wysel_d[i * 128 : (i + 1) * 128, :])
            wy_sb.append(t)

        psums = []
        for i in range(BT * HH):
            psums.append(pspool.tile([128, 512], F32, tag=f"ps{i}", name=f"ps{i}"))

        for kc in range(KT):
            ks = slice(kc * 128, (kc + 1) * 128)
            wh = wpool.tile([128, H], BF16, tag="wh", name="wh")
            nc.sync.dma_start(out=wh, in_=wht_d[ks, :])
            ah = apool.tile([128, BS], BF16, tag="ah", name="ah")
            nc.sync.dma_start(out=ah, in_=aht_d[ks, :])
            wl = wpool.tile([128, H], BF16, tag="wl", name="wl")
            nc.sync.dma_start(out=wl, in_=wlt_d[ks, :])
            al = apool.tile([128, BS], BF16, tag="al", name="al")
            nc.sync.dma_start(out=al, in_=alt_d[ks, :])
            start = kc == 0
            stop = kc == KT - 1
            for bt_i in range(BT):
                ah_s = ah[:, bt_i * 128 : (bt_i + 1) * 128]
                al_s = al[:, bt_i * 128 : (bt_i + 1) * 128]
                for hh_i in range(HH):
                    whs = wh[:, hh_i * 512 : (hh_i + 1) * 512]
                    nc.tensor.matmul(
                        psums[bt_i * HH + hh_i], ah_s, whs, start=start, stop=False
                    )
                for hh_i in range(HH):
                    wls = wl[:, hh_i * 512 : (hh_i + 1) * 512]
                    nc.tensor.matmul(
                        psums[bt_i * HH + hh_i], ah_s, wls, start=False, stop=False
                    )
                for hh_i in range(HH):
                    whs = wh[:, hh_i * 512 : (hh_i + 1) * 512]
                    nc.tensor.matmul(
                        psums[bt_i * HH + hh_i], al_s, whs, start=False, stop=stop
                    )

        # epilogue: bias add (Vector) + tanh (ACT) + hidden DMA (SP queue)
        zb_sb = []
        for i in range(BT * HH):
            bt_i, hh_i = divmod(i, HH)
            zb = epool.tile([128, 512], F32, tag=f"zb{i}", name=f"zb{i}")
            nc.vector.tensor_tensor(
                zb, psums[i], bias_sb[:, hh_i * 512 : (hh_i + 1) * 512], OP.add
            )
            zb_sb.append(zb)
        hid_sb = []
        for i in range(BT * HH):
            hsb = epool.tile([128, 512], F32, tag=f"hid{i}", name=f"hid{i}")
            nc.scalar.activation(hsb, zb_sb[i], AF.Tanh)
            hid_sb.append(hsb)
        for i in range(BT * HH):
            bt_i, hh_i = divmod(i, HH)
            nc.sync.dma_start(
                out=hid_d[
                    bt_i * 128 : (bt_i + 1) * 128, hh_i * 512 : (hh_i + 1) * 512
                ],
                in_=hid_sb[i],
            )

        # head: per-row dot(hidden, wy_sel) -> zsum (logit sans b_y; host
        # finishes sigmoid/BCE). Mults on GpSimd (SBUF-only operands),
        # reduces split Vector / ACT-accum.
        zsum_sb = epool.tile([128, BT], F32, name="zsum_sb")
        scr_sb = []
        for bt_i in range(BT):
            scr = epool.tile([128, H], F32, tag=f"scr{bt_i}", name=f"scr{bt_i}")
            for hh_i in range(HH):
                nc.gpsimd.tensor_tensor(
                    scr[:, hh_i * 512 : (hh_i + 1) * 512],
                    hid_sb[bt_i * HH + hh_i],
                    wy_sb[bt_i][:, hh_i * 512 : (hh_i + 1) * 512],
                    OP.mult,
                )
            scr_sb.append(scr)
        dump0 = epool.tile([128, H], F32, name="dump0")
        dump1 = epool.tile([128, H], F32, name="dump1")
        for bt_i in range(BT):
            if bt_i % 2 == 0:
                nc.vector.tensor_reduce(
                    zsum_sb[:, bt_i : bt_i + 1],
                    scr_sb[bt_i],
                    mybir.AxisListType.X,
                    OP.add,
                )
            else:
                nc.scalar.activation(
                    dump0 if bt_i == 1 else dump1,
                    scr_sb[bt_i],
                    AF.Copy,
                    accum_out=zsum_sb[:, bt_i : bt_i + 1],
                )
        nc.scalar.dma_start(out=zsum_d[:, :], in_=zsum_sb)

    nc.finalize()
    _built = nc
    return nc


def kernel(state, inputX, inputY, truth, W_t, b_t, W_x, b_x, W_y, b_y):
    global LAST_RESULT
    bf16 = ml_dtypes.bfloat16
    f32 = np.float32

    A = np.concatenate([np.asarray(state, f32), np.asarray(inputX, f32)], axis=1)
    Ah = A.astype(bf16)
    Al = (A - Ah.astype(f32)).astype(bf16)

    Wcat = np.concatenate([np.asarray(W_t, f32), np.asarray(W_x, f32)], axis=1)
    Wh = Wcat.astype(bf16)
    Wl = (Wcat - Wh.astype(f32)).astype(bf16)
    WhT = np.ascontiguousarray(Wh.T)
    WlT = np.ascontiguousarray(Wl.T)

    biasb = np.ascontiguousarray(
        np.broadcast_to((np.asarray(b_t, f32) + np.asarray(b_x, f32)), (128, H))
    )

    q = np.argmax(np.asarray(inputY), axis=1)
    Wy_sel = np.ascontiguousarray(np.asarray(W_y, f32)[q])  # [B, H]
    by_sel = np.asarray(b_y, f32)[q]                        # [B]
    truth_f = np.asarray(truth, np.float64)

    in_maps = []
    for c in range(NCORES):
        sl = slice(c * BS, (c + 1) * BS)
        in_maps.append(
            {
                "aht": np.ascontiguousarray(Ah[sl].T),
                "alt": np.ascontiguousarray(Al[sl].T),
                "wht": WhT,
                "wlt": WlT,
                "biasb": biasb,
                "wysel": Wy_sel[sl],
            }
        )

    nc = _build()
    res = run_bass_kernel_spmd(nc, in_maps, core_ids=list(range(NCORES)))
    LAST_RESULT = res

    hidden = np.empty((B, H), f32)
    zy = np.empty(B, f32)
    for c, out in enumerate(res.results):
        sl = slice(c * BS, (c + 1) * BS)
        hidden[sl] = out["hidden_out"]
        zy[sl] = np.asarray(out["zsum"]).T.reshape(BS)
    zy = zy + by_sel
    pred = (1.0 / (1.0 + np.exp(-zy.astype(np.float64)))).astype(f32)
    p64 = pred.astype(np.float64)
    lp = np.maximum(np.log(p64), -100.0)
    l1 = np.maximum(np.log1p(-p64), -100.0)
    bce = truth_f * lp + (1.0 - truth_f) * l1
    err = f32(-bce.sum())
    return pred, err, hidden


# revision 6
# speedup vs baseline: 1.0683x; 1.0349x over previous
import sys

sys.path.insert(0, "/opt/trn_rl_repo")

import numpy as np
import ml_dtypes

import concourse.bass as bass
import concourse.tile as tile
import concourse.mybir as mybir
from concourse import bacc
from concourse.bass_utils import run_bass_kernel_spmd

B, H, NI, NQ = 4096, 1024, 4096, 2048
NCORES = 8
BS = B // NCORES      # 512 batch rows per core
K = H + NI            # 5120 contraction dim
KT = K // 128         # 40 k-tiles
BT = BS // 128        # 4 batch subtiles per core
HH = H // 512         # 2 h-half passes

F32 = mybir.dt.float32
BF16 = mybir.dt.bfloat16
AF = mybir.ActivationFunctionType
OP = mybir.AluOpType

_built = None
LAST_RESULT = None


def _build():
    global _built
    if _built is not None:
        return _built

    nc = bacc.Bacc()
    aht_d = nc.dram_tensor("aht", [K, BS], BF16, kind="ExternalInput")
    alt_d = nc.dram_tensor("alt", [K, BS], BF16, kind="ExternalInput")
    wht_d = nc.dram_tensor("wht", [K, H], BF16, kind="ExternalInput")
    wlt_d = nc.dram_tensor("wlt", [K, H], BF16, kind="ExternalInput")
    biasb_d = nc.dram_tensor("biasb", [128, H], F32, kind="ExternalInput")
    wysel_d = nc.dram_tensor("wysel", [BS, H], F32, kind="ExternalInput")
    hid_d = nc.dram_tensor("hidden_out", [BS, H], F32, kind="ExternalOutput")
    zsum_d = nc.dram_tensor("zsum", [128, BT], F32, kind="ExternalOutput")

    from contextlib import ExitStack

    with tile.TileContext(nc) as tc, ExitStack() as ctx:
        wpool = ctx.enter_context(tc.tile_pool(name="wpool", bufs=4))
        apool = ctx.enter_context(tc.tile_pool(name="apool", bufs=4))
        cpool = ctx.enter_context(tc.tile_pool(name="cpool", bufs=1))
        epool = ctx.enter_context(tc.tile_pool(name="epool", bufs=1))
        pspool = ctx.enter_context(tc.tile_pool(name="pspool", bufs=2, space="PSUM"))

        bias_sb = cpool.tile([128, H], F32, name="bias_sb")
        wy_sb = [
            cpool.tile([128, H], F32, tag=f"wy{i}", name=f"wy{i}") for i in range(BT)
        ]
        zsum_p = [
            epool.tile([128, BT], F32, tag=f"zsp{p}", name=f"zsp{p}") for p in range(HH)
        ]
        zsum_sb = epool.tile([128, BT], F32, name="zsum_sb")
        zb_sb = [None] * (HH * BT)
        hid_sb = [None] * (HH * BT)
        psums_p = {}

        def epilogue(p):
            psums = psums_p[p]
            # bias add (Vector) + tanh (ACT) + dot-mult (Vector) +
            # half-reduce (pass0: Vector; pass1 tail: split Vector/ACT)
            hs = slice(p * 512, (p + 1) * 512)
            for bt_i in range(BT):
                zb = epool.tile([128, 512], F32, tag=f"zb{p}_{bt_i}", name=f"zb{p}_{bt_i}")
                nc.vector.tensor_tensor(zb, psums[bt_i], bias_sb[:, hs], OP.add)
                zb_sb[p * BT + bt_i] = zb
            for bt_i in range(BT):
                hsb = epool.tile(
                    [128, 512], F32, tag=f"hid{p}_{bt_i}", name=f"hid{p}_{bt_i}"
                )
                nc.scalar.activation(hsb, zb_sb[p * BT + bt_i], AF.Tanh)
                hid_sb[p * BT + bt_i] = hsb
            scrs = []
            for bt_i in range(BT):
                scr = epool.tile(
                    [128, 512], F32, tag=f"scr{p}_{bt_i}", name=f"scr{p}_{bt_i}"
                )
                nc.vector.tensor_tensor(
                    scr, hid_sb[p * BT + bt_i], wy_sb[bt_i][:, hs], OP.mult
                )
                scrs.append(scr)
            for bt_i in range(BT):
                zcol = zsum_p[p][:, bt_i : bt_i + 1]
                if p == 0 or bt_i % 2 == 0:
                    nc.vector.tensor_reduce(
                        zcol, scrs[bt_i], mybir.AxisListType.X, OP.add
                    )
                else:
                    dump = epool.tile(
                        [128, 512], F32, tag=f"dump{bt_i}", name=f"dump{bt_i}"
                    )
                    nc.scalar.activation(dump, scrs[bt_i], AF.Copy, accum_out=zcol)

        for p in range(HH):
            hs = slice(p * 512, (p + 1) * 512)
            psums = [
                pspool.tile([128, 512], F32, tag=f"ps{i}", name=f"ps{p}_{i}")
                for i in range(BT)
            ]
            psums_p[p] = psums
            for kc in range(KT):
                ks = slice(kc * 128, (kc + 1) * 128)
                # W halves stream on SP queue, A tiles on ACT queue
                wh = wpool.tile([128, 512], BF16, tag="wh", name="wh")
                nc.sync.dma_start(out=wh, in_=wht_d[ks, hs])
                ah = apool.tile([128, BS], BF16, tag="ah", name="ah")
                nc.scalar.dma_start(out=ah, in_=aht_d[ks, :])
                wl = wpool.tile([128, 512], BF16, tag="wl", name="wl")
                nc.sync.dma_start(out=wl, in_=wlt_d[ks, hs])
                al = apool.tile([128, BS], BF16, tag="al", name="al")
                nc.scalar.dma_start(out=al, in_=alt_d[ks, :])

                if p == 0:
                    # persistent loads injected into the ACT queue early,
                    # spread out so the a-tile prefetch never starves
                    if kc == 2:
                        nc.scalar.dma_start(out=bias_sb, in_=biasb_d[:, :])
                    if kc in (4, 10, 16, 22):
                        i = (kc - 4) // 6
                        nc.scalar.dma_start(
                            out=wy_sb[i], in_=wysel_d[i * 128 : (i + 1) * 128, :]
                        )
                else:
                    # pass-0 epilogue compute emitted after pass-1 kc0
                    # prefetch triggers; hidden writeback of pass 0 spread
                    # over the SP queue during pass 1
                    if kc == 1:
                        epilogue(0)
                    if kc in (3, 5, 7, 9):
                        i = (kc - 3) // 2
                        nc.sync.dma_start(
                            out=hid_d[i * 128 : (i + 1) * 128, 0:512],
                            in_=hid_sb[i],
                        )

                start = kc == 0
                stop = kc == KT - 1
                for bt_i in range(BT):
                    ah_s = ah[:, bt_i * 128 : (bt_i + 1) * 128]
                    al_s = al[:, bt_i * 128 : (bt_i + 1) * 128]
                    nc.tensor.matmul(psums[bt_i], ah_s, wh, start=start, stop=False)
                    nc.tensor.matmul(psums[bt_i], ah_s, wl, start=False, stop=False)
                    nc.tensor.matmul(psums[bt_i], al_s, wh, start=False, stop=stop)

        epilogue(1)
        nc.vector.tensor_tensor(zsum_sb, zsum_p[0], zsum_p[1], OP.add)
        # pass-1 hidden writeback split across both DMA queues
        for bt_i in range(BT):
            eng = nc.sync if bt_i % 2 == 0 else nc.scalar
            eng.dma_start(
                out=hid_d[bt_i * 128 : (bt_i + 1) * 128, 512:1024],
                in_=hid_sb[BT + bt_i],
            )
        nc.scalar.dma_start(out=zsum_d[:, :], in_=zsum_sb)

    nc.finalize()
    _built = nc
    return nc


def kernel(state, inputX, inputY, truth, W_t, b_t, W_x, b_x, W_y, b_y):
    global LAST_RESULT
    bf16 = ml_dtypes.bfloat16
    f32 = np.float32

    A = np.concatenate([np.asarray(state, f32), np.asarray(inputX, f32)], axis=1)
    Ah = A.astype(bf16)
    Al = (A - Ah.astype(f32)).astype(bf16)

    Wcat = np.concatenate([np.asarray(W_t, f32), np.asarray(W_x, f32)], axis=1)
    Wh = Wcat.astype(bf16)
    Wl = (Wcat - Wh.astype(f32)).astype(bf16)
    WhT = np.ascontiguousarray(Wh.T)
    WlT = np.ascontiguousarray(Wl.T)

    biasb = np.ascontiguousarray(
        np.broadcast_to((np.asarray(b_t, f32) + np.asarray(b_x, f32)), (128, H))
    )

    q = np.argmax(np.asarray(inputY), axis=1)
    Wy_sel = np.ascontiguousarray(np.asarray(W_y, f32)[q])  # [B, H]
    by_sel = np.asarray(b_y, f32)[q]                        # [B]
    truth_f = np.asarray(truth, np.float64)

    in_maps = []
    for c in range(NCORES):
        sl = slice(c * BS, (c + 1) * BS)
        in_maps.append(
            {
                "aht": np.ascontiguousarray(Ah[sl].T),
                "alt": np.ascontiguousarray(Al[sl].T),
                "wht": WhT,
                "wlt": WlT,
                "biasb": biasb,
                "wysel": Wy_sel[sl],
            }
        )

    nc = _build()
    res = run_bass_kernel_spmd(nc, in_maps, core_ids=list(range(NCORES)))
    LAST_RESULT = res

    hidden = np.empty((B, H), f32)
    zy = np.empty(B, f32)
    for c, out in enumerate(res.results):
        sl = slice(c * BS, (c + 1) * BS)
        hidden[sl] = out["hidden_out"]
        zy[sl] = np.asarray(out["zsum"]).T.reshape(BS)
    zy = zy + by_sel
    pred = (1.0 / (1.0 + np.exp(-zy.astype(np.float64)))).astype(f32)
    p64 = pred.astype(np.float64)
    lp = np.maximum(np.log(p64), -100.0)
    l1 = np.maximum(np.log1p(-p64), -100.0)
    bce = truth_f * lp + (1.0 - truth_f) * l1
    err = f32(-bce.sum())
    return pred, err, hidden


# revision 12
# speedup vs baseline: 1.0745x; 1.0058x over previous
import sys

sys.path.insert(0, "/opt/trn_rl_repo")

import numpy as np
import ml_dtypes

import concourse.bass as bass
import concourse.tile as tile
import concourse.mybir as mybir
from concourse import bacc
from concourse.bass_utils import run_bass_kernel_spmd

B, H, NI, NQ = 4096, 1024, 4096, 2048
NCORES = 8
BS = B // NCORES      # 512 batch rows per core
K = H + NI            # 5120 contraction dim
KT = K // 128         # 40 k-tiles
BT = BS // 128        # 4 batch subtiles per core
HH = H // 512         # 2 h-half passes

F32 = mybir.dt.float32
BF16 = mybir.dt.bfloat16
AF = mybir.ActivationFunctionType
OP = mybir.AluOpType

_built = None
LAST_RESULT = None


def _build():
    global _built
    if _built is not None:
        return _built

    nc = bacc.Bacc()
    aht_d = nc.dram_tensor("aht", [K, BS], BF16, kind="ExternalInput")
    alt_d = nc.dram_tensor("alt", [K, BS], BF16, kind="ExternalInput")
    wht_d = nc.dram_tensor("wht", [K, H], BF16, kind="ExternalInput")
    wlt_d = nc.dram_tensor("wlt", [K, H], BF16, kind="ExternalInput")
    biasb_d = nc.dram_tensor("biasb", [128, H], F32, kind="ExternalInput")
    wysel_d = nc.dram_tensor("wysel", [BS, H], F32, kind="ExternalInput")
    hid_d = nc.dram_tensor("hidden_out", [BS, H], F32, kind="ExternalOutput")
    zsum_d = nc.dram_tensor("zsum", [128, BT], F32, kind="ExternalOutput")

    from contextlib import ExitStack

    with tile.TileContext(nc) as tc, ExitStack() as ctx:
        wpool = ctx.enter_context(tc.tile_pool(name="wpool", bufs=4))
        apool = ctx.enter_context(tc.tile_pool(name="apool", bufs=4))
        cpool = ctx.enter_context(tc.tile_pool(name="cpool", bufs=1))
        epool = ctx.enter_context(tc.tile_pool(name="epool", bufs=1))
        pspool = ctx.enter_context(tc.tile_pool(name="pspool", bufs=2, space="PSUM"))

        bias_sb = cpool.tile([128, H], F32, name="bias_sb")
        wy_sb = [
            cpool.tile([128, H], F32, tag=f"wy{i}", name=f"wy{i}") for i in range(BT)
        ]
        zsum_p = [
            epool.tile([128, BT], F32, tag=f"zsp{p}", name=f"zsp{p}") for p in range(HH)
        ]
        zsum_sb = epool.tile([128, BT], F32, name="zsum_sb")
        zb_sb = [None] * (HH * BT)
        hid_sb = [None] * (HH * BT)
        psums_p = {}

        def epilogue(p):
            psums = psums_p[p]
            # bias add (Vector) + tanh (ACT) + dot-mult (Vector) +
            # half-reduce (pass0: Vector; pass1 tail: split Vector/ACT)
            hs = slice(p * 512, (p + 1) * 512)
            for bt_i in range(BT):
                zb = epool.tile([128, 512], F32, tag=f"zb{p}_{bt_i}", name=f"zb{p}_{bt_i}")
                nc.vector.tensor_tensor(zb, psums[bt_i], bias_sb[:, hs], OP.add)
                zb_sb[p * BT + bt_i] = zb
            for bt_i in range(BT):
                hsb = epool.tile(
                    [128, 512], F32, tag=f"hid{p}_{bt_i}", name=f"hid{p}_{bt_i}"
                )
                nc.scalar.activation(hsb, zb_sb[p * BT + bt_i], AF.Tanh)
                hid_sb[p * BT + bt_i] = hsb
            scrs = []
            for bt_i in range(BT):
                scr = epool.tile(
                    [128, 512], F32, tag=f"scr{p}_{bt_i}", name=f"scr{p}_{bt_i}"
                )
                nc.vector.tensor_tensor(
                    scr, hid_sb[p * BT + bt_i], wy_sb[bt_i][:, hs], OP.mult
                )
                scrs.append(scr)
            for bt_i in range(BT):
                zcol = zsum_p[p][:, bt_i : bt_i + 1]
                if p == 0 or bt_i % 2 == 0:
                    nc.vector.tensor_reduce(
                        zcol, scrs[bt_i], mybir.AxisListType.X, OP.add
                    )
                else:
                    dump = epool.tile(
                        [128, 512], F32, tag=f"dump{bt_i}", name=f"dump{bt_i}"
                    )
                    nc.scalar.activation(dump, scrs[bt_i], AF.Copy, accum_out=zcol)

        for p in range(HH):
            hs = slice(p * 512, (p + 1) * 512)
            psums = [
                pspool.tile([128, 512], F32, tag=f"ps{i}", name=f"ps{p}_{i}")
                for i in range(BT)
            ]
            psums_p[p] = psums
            for kc in range(KT):
                ks = slice(kc * 128, (kc + 1) * 128)
                # W halves stream on SP queue, A tiles on ACT queue
                wh = wpool.tile([128, 512], BF16, tag="wh", name="wh")
                nc.sync.dma_start(out=wh, in_=wht_d[ks, hs])
                ah = apool.tile([128, BS], BF16, tag="ah", name="ah")
                nc.scalar.dma_start(out=ah, in_=aht_d[ks, :])
                wl = wpool.tile([128, 512], BF16, tag="wl", name="wl")
                nc.sync.dma_start(out=wl, in_=wlt_d[ks, hs])
                al = apool.tile([128, BS], BF16, tag="al", name="al")
                nc.scalar.dma_start(out=al, in_=alt_d[ks, :])

                if p == 0:
                    # persistent loads injected into the ACT queue early,
                    # spread out so the a-tile prefetch never starves
                    if kc == 2:
                        nc.scalar.dma_start(out=bias_sb, in_=biasb_d[:, :])
                    if kc in (4, 10, 16, 22):
                        i = (kc - 4) // 6
                        nc.scalar.dma_start(
                            out=wy_sb[i], in_=wysel_d[i * 128 : (i + 1) * 128, :]
                        )
                else:
                    # pass-0 epilogue compute emitted after pass-1 kc0
                    # prefetch triggers; hidden writeback of pass 0 spread
                    # over the SP queue during pass 1
                    if kc == 1:
                        epilogue(0)
                    if kc in (3, 5, 7, 9):
                        i = (kc - 3) // 2
                        nc.sync.dma_start(
                            out=hid_d[i * 128 : (i + 1) * 128, 0:512],
                            in_=hid_sb[i],
                        )

                start = kc == 0
                stop = kc == KT - 1
                for bt_i in range(BT):
                    ah_s = ah[:, bt_i * 128 : (bt_i + 1) * 128]
                    al_s = al[:, bt_i * 128 : (bt_i + 1) * 128]
                    nc.tensor.matmul(psums[bt_i], ah_s, wh, start=start, stop=False)
                    nc.tensor.matmul(psums[bt_i], ah_s, wl, start=False, stop=False)
                    nc.tensor.matmul(psums[bt_i], al_s, wh, start=False, stop=stop)

        epilogue(1)
        nc.vector.tensor_tensor(zsum_sb, zsum_p[0], zsum_p[1], OP.add)
        # pass-1 hidden writeback split across both DMA queues
        for bt_i in range(BT):
            eng = nc.sync if bt_i % 2 == 0 else nc.scalar
            eng.dma_start(
                out=hid_d[bt_i * 128 : (bt_i + 1) * 128, 512:1024],
                in_=hid_sb[BT + bt_i],
            )
        nc.scalar.dma_start(out=zsum_d[:, :], in_=zsum_sb)

    nc.finalize()
    _built = nc
    return nc


def kernel(state, inputX, inputY, truth, W_t, b_t, W_x, b_x, W_y, b_y):
    global LAST_RESULT
    bf16 = ml_dtypes.bfloat16
    f32 = np.float32

    A = np.concatenate([np.asarray(state, f32), np.asarray(inputX, f32)], axis=1)
    Ah = A.astype(bf16)
    Al = (A - Ah.astype(f32)).astype(bf16)

    Wcat = np.concatenate([np.asarray(W_t, f32), np.asarray(W_x, f32)], axis=1)
    Wh = Wcat.astype(bf16)
    Wl = (Wcat - Wh.astype(f32)).astype(bf16)
    WhT = np.ascontiguousarray(Wh.T)
    WlT = np.ascontiguousarray(Wl.T)

    biasb = np.ascontiguousarray(
        np.broadcast_to((np.asarray(b_t, f32) + np.asarray(b_x, f32)), (128, H))
    )

    q = np.argmax(np.asarray(inputY), axis=1)
    Wy_sel = np.ascontiguousarray(np.asarray(W_y, f32)[q])  # [B, H]
    by_sel = np.asarray(b_y, f32)[q]                        # [B]
    truth_f = np.asarray(truth, np.float64)

    in_maps = []
    for c in range(NCORES):
        sl = slice(c * BS, (c + 1) * BS)
        in_maps.append(
            {
                "aht": np.ascontiguousarray(Ah[sl].T),
                "alt": np.ascontiguousarray(Al[sl].T),
                "wht": WhT,
                "wlt": WlT,
                "biasb": biasb,
                "wysel": Wy_sel[sl],
            }
        )

    nc = _build()
    res = run_bass_kernel_spmd(nc, in_maps, core_ids=list(range(NCORES)))
    LAST_RESULT = res

    hidden = np.empty((B, H), f32)
    zy = np.empty(B, f32)
    for c, out in enumerate(res.results):
        sl = slice(c * BS, (c + 1) * BS)
        hidden[sl] = out["hidden_out"]
        zy[sl] = np.asarray(out["zsum"]).T.reshape(BS)
    zy = zy + by_sel
    pred = (1.0 / (1.0 + np.exp(-zy.astype(np.float64)))).astype(f32)
    p64 = pred.astype(np.float64)
    lp = np.maximum(np.log(p64), -100.0)
    l1 = np.maximum(np.log1p(-p64), -100.0)
    bce = truth_f * lp + (1.0 - truth_f) * l1
    err = f32(-bce.sum())
    return pred, err, hidden
